# revision 3
# baseline (speedup 1.0000x reference)
"""Trainium2 Bass kernel for the adaptive semantic-scal loss (segment_reduce).

Self-contained: hardcodes shapes/sharding for
  pred [2,17,200,200,16] f32, ssc_target [2,200,200,16] int, f1_list [17] f32.

v2 strategy (8 NeuronCores, data-parallel over voxels; 160k voxels/core laid
out as 128 partitions x 1250 voxels, slab-major / class-major within each
partition; every 125-voxel chunk gets a leading "gap" column):

  DMA:  pred ships as fp8_e4m3; 5 slab DMAs on the Sync HWDGE ring, tgt on
        the Scalar HWDGE ring (fast issue, frees GpSimd for compute).
  ACT:  E = exp(pred) per slab; at the end, 5 per-bank PSUM->SBUF copies
        (extraction moved off DVE).
  DVE:  onehot (17 classes, 4x mode, overlaps DMA ramp); per slab:
        class-tree-sum -> S, fast reciprocal straight to bf16 -> W,
        R = E*W in place, gap columns of R set to 1.
  GPSIMD: full tree+mult for the chunk-0 slab (offloads the DVE).
  PE:   per class c (17) per chunk: psum += OH_chunk^T @ R_chunk; classes
        map 4-per-bank (banks 0..3, 126-col regions) + class 16 in bank 4;
        one accumulation chain per bank. Gap columns make row 0 = sum_p
        partials and col 0 = count partials; the diagonal holds nominator
        partials.
  out:  raw [126, 4*504+126] bf16 partial tile per core; host does the
        diag/row0/col0 extraction, the cross-core sums, and the 17-element
        scalar loss epilogue in numpy.
"""

import sys

for _p in ("/opt/trn_rl_repo",):
    if _p not in sys.path:
        sys.path.append(_p)

import numpy as np
import ml_dtypes

import concourse.bacc as bacc
import concourse.tile as tile
import concourse.mybir as mybir
from concourse.bass_utils import run_bass_kernel_spmd

F32 = mybir.dt.float32
BF16 = mybir.dt.bfloat16
FP8 = mybir.dt.float8e4
ALU = mybir.AluOpType
ACTF = mybir.ActivationFunctionType

N_CORES = 8
P = 128          # partitions
C = 17           # classes
KV = 1250        # real voxels per partition per core (128*1250*8 = 1.28M)
W = 125          # data voxels per matmul chunk
WP = W + 1       # chunk width incl. leading ones-gap column
NCH = 10         # chunks per partition
KVP = NCH * WP   # padded voxels per partition (1260)
CPSL = [1, 2, 3, 3, 1]            # chunks per DMA/exp slab
NSLAB = len(CPSL)
COFF = [sum(CPSL[:i]) for i in range(NSLAB)]   # chunk offset per slab

# work slabs: (chunk_offset, n_chunks, engine)  engine: 'g' gpsimd, 'v' dve
WSLAB = [(0, 1, 'g'), (1, 2, 'v'), (3, 3, 'v'), (6, 3, 'v'), (9, 1, 'v')]

# class -> (psum bank, col region)
def _bankreg(c):
    return (c // 4, c % 4) if c < 16 else (4, 0)

OUTW = 4 * 504 + 126   # 2142 bf16 cols in the output tile

BETA = 0.95
ALPHA = 5.0
WPC = 3.0
NTOT = float(N_CORES * P * KV)  # all targets are valid (0..16)


def _build():
    nc = bacc.Bacc("TRN2", target_bir_lowering=False, debug=False,
                   num_devices=N_CORES)
    pred_d = nc.dram_tensor("pred", [P, C * KVP], FP8, kind="ExternalInput")
    tgt_d = nc.dram_tensor("tgt", [P, KVP], BF16, kind="ExternalInput")
    out_d = nc.dram_tensor("out", [WP, OUTW], BF16, kind="ExternalOutput")

    with tile.TileContext(nc) as tc:
        with (
            tc.tile_pool(name="pred", bufs=1) as pk,
            tc.tile_pool(name="work", bufs=1) as pw,
            tc.tile_pool(name="small", bufs=2) as ps,
            tc.tile_pool(name="persist", bufs=1) as pa,
            tc.tile_pool(name="psum", bufs=1, space="PSUM") as pp,
        ):
            # ---- input DMAs: HWDGE rings (sync for pred, scalar for tgt) --
            tgt_sb = pa.tile([P, KVP], BF16)
            nc.scalar.dma_start(out=tgt_sb[:, :], in_=tgt_d[:, :])

            pred_sb = pk.tile([P, C, KVP], FP8)   # slab-major runs
            for s in range(NSLAB):
                a = C * WP * COFF[s]
                b = C * WP * (COFF[s] + CPSL[s])
                nc.sync.dma_start(
                    out=pred_sb[:, :, :].rearrange("p c k -> p (c k)")[:, a:b],
                    in_=pred_d[:, a:b])

            # slab views: [P, C, w] starting at slab-major flat offsets
            def slab_view(tile_, coff, nch):
                flat = tile_[:, :, :].rearrange("p c k -> p (c k)")
                a = C * WP * coff
                w = WP * nch
                return flat[:, a:a + C * w].rearrange(
                    "p (c k) -> p c k", c=C)

            ER = pw.tile([P, C, KVP], BF16)        # E, then R in place
            OH = pa.tile([P, C, KVP], BF16)        # onehot incl. class 16
            Wt = pa.tile([P, KVP], BF16)           # 1/S per voxel
            out_sb = pa.tile([P, OUTW], BF16)

            # ---- ACT: exp per DMA slab --------------------------------
            for s in range(NSLAB):
                a = C * WP * COFF[s]
                b = C * WP * (COFF[s] + CPSL[s])
                nc.scalar.activation(
                    ER[:, :, :].rearrange("p c k -> p (c k)")[:, a:b],
                    pred_sb[:, :, :].rearrange("p c k -> p (c k)")[:, a:b],
                    ACTF.Exp)

            # ---- DVE: onehot prebuild (overlaps DMA/exp ramp) ---------
            for c in range(C):
                nc.vector.tensor_scalar(OH[:, c, :], tgt_sb[:, :],
                                        float(c), None, ALU.is_equal)
            # gap columns of OH -> 1 (row-0 sum_p trick)
            nc.vector.memset(
                OH[:, :, :].rearrange("p c (g k) -> p c g k",
                                      g=NCH)[:, :, :, 0], 1.0)

            # ---- per work-slab: tree-sum -> S -> W -> R = E*W ---------
            def emit_slab(coff, nch, engine):
                w = WP * nch
                eng = nc.gpsimd if engine == 'g' else nc.vector
                tag = engine
                T8 = ps.tile([P, 8, w], BF16, name="t8_%d%s" % (coff, tag),
                             tag="t8" + tag, bufs=2)
                S = ps.tile([P, w], F32, name="s_%d%s" % (coff, tag),
                            tag="s" + tag, bufs=2)
                e = slab_view(ER, coff, nch)
                eng.tensor_add(T8[:, :, :], e[:, 0:8, :], e[:, 8:16, :])
                eng.tensor_add(T8[:, 0:4, :], T8[:, 0:4, :], T8[:, 4:8, :])
                eng.tensor_add(T8[:, 0:2, :], T8[:, 0:2, :], T8[:, 2:4, :])
                eng.tensor_add(T8[:, 0, :], T8[:, 0, :], T8[:, 1, :])
                eng.tensor_add(S[:, :], T8[:, 0, :], e[:, 16, :])
                # fast reciprocal (fp32-only custom op), then cast to bf16
                Wf = ps.tile([P, w], F32, name="wf_%d%s" % (coff, tag),
                             tag="wf" + tag, bufs=2)
                nc.vector.reciprocal_approx_fast(Wf[:, :], S[:, :])
                wv = Wt[:, WP * coff:WP * coff + w]
                nc.vector.tensor_copy(wv, Wf[:, :])
                wb = wv.rearrange("p (a k) -> p a k", a=1) \
                    .to_broadcast((P, C, w))
                eng.tensor_tensor(e[:, :, :], e[:, :, :], wb, op=ALU.mult)
                # gap columns of R -> 1 (col-0 count trick), all 17 classes
                eng.memset(
                    e[:, :, :].rearrange("p c (g k) -> p c g k",
                                         g=nch)[:, :, :, 0], 1.0)

            for coff, nch, engine in WSLAB:
                emit_slab(coff, nch, engine)

            # ---- PE: 5 banks, one accumulation chain per bank ---------
            pnom = pp.tile([128, 8, 512], F32)
            for coff, nch, engine in WSLAB:
                e = slab_view(ER, coff, nch)
                for h in range(nch):
                    g = coff + h
                    for c in range(C):
                        bank, reg = _bankreg(c)
                        off = 126 * reg
                        nc.tensor.matmul(
                            pnom[0:WP, bank, off:off + WP],
                            OH[:, c, g * WP:(g + 1) * WP],
                            e[:, c, h * WP:(h + 1) * WP],
                            start=(g == 0 and reg == 0),
                            stop=(g == NCH - 1 and (reg == 3 or c == 16)),
                            skip_group_check=True)

            # ---- extraction: ACT copies psum -> sbuf, then DMA out ----
            for bank in range(5):
                w = 504 if bank < 4 else 126
                a = 504 * bank
                nc.scalar.copy(out_sb[0:WP, a:a + w],
                               pnom[0:WP, bank, 0:w])
                nc.sync.dma_start(out=out_d[:, a:a + w],
                                  in_=out_sb[0:WP, a:a + w])

    nc.compile()
    return nc


_NC_CACHE = None


def _get_nc():
    global _NC_CACHE
    if _NC_CACHE is None:
        _NC_CACHE = _build()
    return _NC_CACHE


def _shard_inputs(pred, ssc_target, f1_list=None):
    pred = np.asarray(pred, dtype=np.float32)
    tgt = np.asarray(ssc_target)

    nvox = N_CORES * P * KV
    assert nvox == pred.size // C
    # voxel-major [v, c], then block: [core, p, c, kv]
    pv = np.ascontiguousarray(
        pred.reshape(2, C, -1).transpose(0, 2, 1).reshape(nvox, C)
        .reshape(N_CORES, P, KV, C).transpose(0, 1, 3, 2))
    tv = tgt.reshape(nvox).reshape(N_CORES, P, KV)
    # pad: each 125-voxel chunk gets a leading gap column
    # (pred=0 -> E=1; tgt=255 -> onehot=0)
    pp_ = np.zeros((N_CORES, P, C, NCH, WP), np.float32)
    pp_[..., 1:] = pv.reshape(N_CORES, P, C, NCH, W)
    # slab-major layout: [core, p, slab, c, slab cols]
    pp_ = pp_.reshape(N_CORES, P, C, KVP)
    parts = []
    for s in range(NSLAB):
        a = WP * COFF[s]
        b = a + WP * CPSL[s]
        parts.append(pp_[:, :, :, a:b].reshape(N_CORES, P, -1))
    pf = np.ascontiguousarray(np.concatenate(parts, axis=2)) \
        .astype(ml_dtypes.float8_e4m3)
    tp = np.full((N_CORES, P, NCH, WP), 255.0, np.float32)
    tp[..., 1:] = tv.reshape(N_CORES, P, NCH, W)
    tp = tp.reshape(N_CORES, P, KVP).astype(ml_dtypes.bfloat16)
    return [{"pred": pf[i], "tgt": tp[i]} for i in range(N_CORES)]


def _postprocess(outs, f1_list):
    """outs: per-core [126, 2142] bf16 raw psum tiles -> scalar loss."""
    a = np.asarray(outs, dtype=np.float64)          # [cores, 126, 2142]
    count = np.zeros(C)
    sum_p = np.zeros(C)
    nom = np.zeros(C)
    ii = np.arange(1, WP)
    for c in range(C):
        bank, reg = _bankreg(c)
        blk = a[:, :, 504 * bank + 126 * reg: 504 * bank + 126 * reg + WP]
        nom[c] = blk[:, ii, ii].sum()
        sum_p[c] = blk[:, 0, 1:].sum()
        count[c] = blk[:, 1:, 0].sum()
    n_mask = NTOT

    f1_list = np.asarray(f1_list, dtype=np.float64)
    has = count > 0
    pm = sum_p > 0
    precision = np.where(pm, nom / np.where(pm, sum_p, 1.0), 0.0)
    recall = np.where(has, nom / np.where(has, count, 1.0), 0.0)
    neg = n_mask - count
    spec_num = (n_mask - sum_p) - (count - nom)
    nmp = neg > 0
    specificity = np.where(nmp, spec_num / np.where(nmp, neg, 1.0), 0.0)

    def bce(x):
        return np.minimum(-np.log(np.maximum(x, 1e-38)), 100.0)

    loss_list = np.where(
        has,
        np.where(pm, bce(precision), 0.0) + bce(recall)
        + np.where(nmp, bce(specificity), 0.0),
        0.0)

    denom = precision + recall
    f1 = np.where(denom > 0, 2.0 * precision * recall
                  / np.where(denom > 0, denom, 1.0), 0.0)
    cur_f1 = np.where(has, f1, 0.0)
    new_f1 = BETA * f1_list + (1.0 - BETA) * cur_f1

    cnt = has.sum()
    sel = loss_list != 0
    logits = np.where(sel, ALPHA * (1.0 - new_f1), -np.inf)
    mx = logits.max()
    ex = np.exp(logits - mx)
    sm = ex / ex.sum()
    weighted = loss_list * (1.0 + WPC * cnt * sm)
    loss = weighted.sum() / (cnt * (1.0 + WPC))
    return np.float32(loss)


def kernel(pred, ssc_target, f1_list):
    nc = _get_nc()
    in_maps = _shard_inputs(pred, ssc_target)
    res = run_bass_kernel_spmd(nc, in_maps, core_ids=list(range(N_CORES)))
    outs = [np.asarray(r["out"], dtype=np.float32) for r in res.results]
    return _postprocess(outs, f1_list).reshape(())


if __name__ == "__main__":
    rng = np.random.default_rng(0)
    pred = rng.standard_normal((2, C, 200, 200, 16), dtype=np.float32)
    tgt = rng.integers(0, C, size=(2, C)).astype(np.int64)  # placeholder
    tgt = rng.integers(0, C, size=(2, 200, 200, 16)).astype(np.int64)
    f1l = np.zeros((C,), np.float32)
    print(kernel(pred, tgt, f1l))


# revision 4
# speedup vs baseline: 1.1634x; 1.1634x over previous
"""Trainium2 Bass kernel for the adaptive semantic-scal loss (segment_reduce).

Self-contained: hardcodes shapes/sharding for
  pred [2,17,200,200,16] f32, ssc_target [2,200,200,16] int, f1_list [17] f32.

v3 strategy (8 NeuronCores, data-parallel over voxels; 160k voxels/core laid
out as 128 partitions x 1250 voxels, slab-major / class-major within each
partition; every 125-voxel chunk gets a leading "gap" column):

  host: builds the onehot (incl. gap cols = 1) in fp8 and ships it next to
        pred (fp8) -- no tgt tensor, no onehot build on device. count[c]
        comes from a host bincount.
  DMA:  pred slabs 0-1 on the Scalar HWDGE ring (earliest exp start);
        pred slabs 2-4 and the 5 onehot slabs interleaved on the Sync ring.
  ACT:  E = exp(pred) per slab; at the end, per-bank PSUM->SBUF copies.
  DVE:  per slab: class-tree-sum -> S (f32), fast reciprocal, cast to bf16,
        R = E*W in place. Nothing else.
  PE:   per class c (17) per chunk: psum += OH_chunk^T @ R_chunk (fp8
        stationary x bf16 moving); classes map 4-per-bank (banks 0..3,
        126-col regions) + class 16 in bank 4; one accumulation chain per
        bank. OH gap col = 1 makes psum row 0 = sum_p partials; the
        diagonal holds nominator partials.
  out:  raw [126, 4*504+126] bf16 partial tile per core; host extracts
        diag/row0, sums across cores, and runs the 17-element scalar loss
        epilogue in numpy.
"""

import sys

for _p in ("/opt/trn_rl_repo",):
    if _p not in sys.path:
        sys.path.append(_p)

import numpy as np
import ml_dtypes

import concourse.bacc as bacc
import concourse.tile as tile
import concourse.mybir as mybir
from concourse.bass_utils import run_bass_kernel_spmd

F32 = mybir.dt.float32
BF16 = mybir.dt.bfloat16
FP8 = mybir.dt.float8e4
ALU = mybir.AluOpType
ACTF = mybir.ActivationFunctionType

N_CORES = 8
P = 128          # partitions
C = 17           # classes
KV = 1250        # real voxels per partition per core (128*1250*8 = 1.28M)
W = 125          # data voxels per matmul chunk
WP = W + 1       # chunk width incl. leading ones-gap column
NCH = 10         # chunks per partition
KVP = NCH * WP   # padded voxels per partition (1260)
CPSL = [1, 2, 3, 3, 1]            # chunks per DMA/exp slab
NSLAB = len(CPSL)
COFF = [sum(CPSL[:i]) for i in range(NSLAB)]   # chunk offset per slab
SCAL_RING_SLABS = 2               # pred slabs issued on the scalar HWDGE ring

# class -> (psum bank, col region)
def _bankreg(c):
    return (c // 4, c % 4) if c < 16 else (4, 0)

OUTW = 4 * 504 + 126   # 2142 bf16 cols in the output tile

BETA = 0.95
ALPHA = 5.0
WPC = 3.0
NTOT = float(N_CORES * P * KV)  # all targets are valid (0..16)


def _build():
    nc = bacc.Bacc("TRN2", target_bir_lowering=False, debug=False,
                   num_devices=N_CORES)
    pred_d = nc.dram_tensor("pred", [P, C * KVP], FP8, kind="ExternalInput")
    oh_d = nc.dram_tensor("oh", [P, C * KVP], FP8, kind="ExternalInput")
    out_d = nc.dram_tensor("out", [WP, OUTW], BF16, kind="ExternalOutput")

    def slab_rng(s):
        return C * WP * COFF[s], C * WP * (COFF[s] + CPSL[s])

    with tile.TileContext(nc) as tc:
        with (
            tc.tile_pool(name="pred", bufs=1) as pk,
            tc.tile_pool(name="work", bufs=1) as pw,
            tc.tile_pool(name="small", bufs=2) as ps,
            tc.tile_pool(name="persist", bufs=1) as pa,
            tc.tile_pool(name="psum", bufs=1, space="PSUM") as pp,
        ):
            pred_sb = pk.tile([P, C, KVP], FP8)   # slab-major runs
            OH = pa.tile([P, C, KVP], FP8)
            pred_fl = pred_sb[:, :, :].rearrange("p c k -> p (c k)")
            oh_fl = OH[:, :, :].rearrange("p c k -> p (c k)")

            # pred slabs 0..1 on the scalar HWDGE ring: earliest exp start
            for s in range(SCAL_RING_SLABS):
                a, b = slab_rng(s)
                nc.scalar.dma_start(out=pred_fl[:, a:b], in_=pred_d[:, a:b])
            # sync ring: remaining pred slabs + onehot slabs, interleaved so
            # pred stays ahead of exp and OH_s lands before its matmuls
            order = [("p", 2), ("o", 0), ("o", 1), ("p", 3), ("o", 2),
                     ("p", 4), ("o", 3), ("o", 4)]
            for kind, s in order:
                a, b = slab_rng(s)
                if kind == "p":
                    nc.sync.dma_start(out=pred_fl[:, a:b], in_=pred_d[:, a:b])
                else:
                    nc.sync.dma_start(out=oh_fl[:, a:b], in_=oh_d[:, a:b])

            def slab_view(tile_, coff, nch):
                flat = tile_[:, :, :].rearrange("p c k -> p (c k)")
                a = C * WP * coff
                w = WP * nch
                return flat[:, a:a + C * w].rearrange(
                    "p (c k) -> p c k", c=C)

            ER = pw.tile([P, C, KVP], BF16)        # E, then R in place
            Wt = pa.tile([P, KVP], BF16)           # 1/S per voxel
            out_sb = pa.tile([P, OUTW], BF16)

            # ---- ACT: exp per slab ------------------------------------
            for s in range(NSLAB):
                a, b = slab_rng(s)
                nc.scalar.activation(
                    ER[:, :, :].rearrange("p c k -> p (c k)")[:, a:b],
                    pred_fl[:, a:b], ACTF.Exp)

            # ---- DVE per slab: tree -> S -> 1/S -> R = E*W ------------
            def emit_slab(s):
                coff, nch = COFF[s], CPSL[s]
                w = WP * nch
                T8 = ps.tile([P, 8, w], BF16, name="t8_%d" % s,
                             tag="t8", bufs=2)
                S = ps.tile([P, w], F32, name="s_%d" % s, tag="s", bufs=2)
                Wf = ps.tile([P, w], F32, name="wf_%d" % s, tag="wf", bufs=2)
                e = slab_view(ER, coff, nch)
                nc.vector.tensor_add(T8[:, :, :], e[:, 0:8, :], e[:, 8:16, :])
                nc.vector.tensor_add(T8[:, 0:4, :], T8[:, 0:4, :],
                                     T8[:, 4:8, :])
                nc.vector.tensor_add(T8[:, 0:2, :], T8[:, 0:2, :],
                                     T8[:, 2:4, :])
                nc.vector.tensor_add(T8[:, 0, :], T8[:, 0, :], T8[:, 1, :])
                nc.vector.tensor_add(S[:, :], T8[:, 0, :], e[:, 16, :])
                nc.vector.reciprocal_approx_fast(Wf[:, :], S[:, :])
                wv = Wt[:, WP * coff:WP * coff + w]
                nc.vector.tensor_copy(wv, Wf[:, :])
                wb = wv.rearrange("p (a k) -> p a k", a=1) \
                    .to_broadcast((P, C, w))
                nc.vector.tensor_tensor(e[:, :, :], e[:, :, :], wb,
                                        op=ALU.mult)

            for s in range(NSLAB):
                emit_slab(s)

            # ---- PE: 5 banks, one accumulation chain per bank ---------
            pnom = pp.tile([128, 8, 512], F32)
            for s in range(NSLAB):
                e = slab_view(ER, COFF[s], CPSL[s])
                o = slab_view(OH, COFF[s], CPSL[s])
                for h in range(CPSL[s]):
                    g = COFF[s] + h
                    for c in range(C):
                        bank, reg = _bankreg(c)
                        off = 126 * reg
                        nc.tensor.matmul(
                            pnom[0:WP, bank, off:off + WP],
                            o[:, c, h * WP:(h + 1) * WP],
                            e[:, c, h * WP:(h + 1) * WP],
                            start=(g == 0 and reg == 0),
                            stop=(g == NCH - 1 and (reg == 3 or c == 16)),
                            skip_group_check=True)

            # ---- extraction: ACT psum->sbuf per bank, 2 DMAs out ------
            for bank in range(5):
                w = 504 if bank < 4 else 126
                a = 504 * bank
                nc.scalar.copy(out_sb[0:WP, a:a + w],
                               pnom[0:WP, bank, 0:w])
            nc.sync.dma_start(out=out_d[:, 0:2016], in_=out_sb[0:WP, 0:2016])
            nc.sync.dma_start(out=out_d[:, 2016:OUTW],
                              in_=out_sb[0:WP, 2016:OUTW])

    nc.compile()
    return nc


_NC_CACHE = None


def _get_nc():
    global _NC_CACHE
    if _NC_CACHE is None:
        _NC_CACHE = _build()
    return _NC_CACHE


def _shard_inputs(pred, ssc_target, f1_list=None):
    pred = np.asarray(pred, dtype=np.float32)
    tgt = np.asarray(ssc_target)

    nvox = N_CORES * P * KV
    assert nvox == pred.size // C
    # voxel-major [v, c], then block: [core, p, c, kv]
    pv = np.ascontiguousarray(
        pred.reshape(2, C, -1).transpose(0, 2, 1).reshape(nvox, C)
        .reshape(N_CORES, P, KV, C).transpose(0, 1, 3, 2))
    tv = tgt.reshape(nvox).reshape(N_CORES, P, KV)
    # onehot [core, p, c, kv]
    ohv = (tv[:, :, None, :] == np.arange(C)[None, None, :, None])

    # pad: each 125-voxel chunk gets a leading gap column
    # (pred gap = 0; onehot gap = 1 -> row-0 sum_p trick)
    def pack(x, gapval, dtype):
        y = np.full((N_CORES, P, C, NCH, WP), gapval, np.float32)
        y[..., 1:] = x.reshape(N_CORES, P, C, NCH, W)
        y = y.reshape(N_CORES, P, C, KVP)
        parts = []
        for s in range(NSLAB):
            a = WP * COFF[s]
            b = a + WP * CPSL[s]
            parts.append(y[:, :, :, a:b].reshape(N_CORES, P, -1))
        return np.ascontiguousarray(np.concatenate(parts, axis=2)) \
            .astype(dtype)

    pf = pack(pv, 0.0, ml_dtypes.float8_e4m3)
    of = pack(ohv.astype(np.float32), 1.0, ml_dtypes.float8_e4m3)
    return [{"pred": pf[i], "oh": of[i]} for i in range(N_CORES)]


def _postprocess(outs, counts, f1_list):
    """outs: per-core [126, 2142] bf16 raw psum tiles -> scalar loss."""
    a = np.asarray(outs, dtype=np.float64)          # [cores, 126, 2142]
    count = counts.astype(np.float64)
    sum_p = np.zeros(C)
    nom = np.zeros(C)
    ii = np.arange(1, WP)
    for c in range(C):
        bank, reg = _bankreg(c)
        blk = a[:, :, 504 * bank + 126 * reg: 504 * bank + 126 * reg + WP]
        nom[c] = blk[:, ii, ii].sum()
        sum_p[c] = blk[:, 0, 1:].sum()
    n_mask = NTOT

    f1_list = np.asarray(f1_list, dtype=np.float64)
    has = count > 0
    pm = sum_p > 0
    precision = np.where(pm, nom / np.where(pm, sum_p, 1.0), 0.0)
    recall = np.where(has, nom / np.where(has, count, 1.0), 0.0)
    neg = n_mask - count
    spec_num = (n_mask - sum_p) - (count - nom)
    nmp = neg > 0
    specificity = np.where(nmp, spec_num / np.where(nmp, neg, 1.0), 0.0)

    def bce(x):
        return np.minimum(-np.log(np.maximum(x, 1e-38)), 100.0)

    loss_list = np.where(
        has,
        np.where(pm, bce(precision), 0.0) + bce(recall)
        + np.where(nmp, bce(specificity), 0.0),
        0.0)

    denom = precision + recall
    f1 = np.where(denom > 0, 2.0 * precision * recall
                  / np.where(denom > 0, denom, 1.0), 0.0)
    cur_f1 = np.where(has, f1, 0.0)
    new_f1 = BETA * f1_list + (1.0 - BETA) * cur_f1

    cnt = has.sum()
    sel = loss_list != 0
    logits = np.where(sel, ALPHA * (1.0 - new_f1), -np.inf)
    mx = logits.max()
    ex = np.exp(logits - mx)
    sm = ex / ex.sum()
    weighted = loss_list * (1.0 + WPC * cnt * sm)
    loss = weighted.sum() / (cnt * (1.0 + WPC))
    return np.float32(loss)


def kernel(pred, ssc_target, f1_list):
    nc = _get_nc()
    in_maps = _shard_inputs(pred, ssc_target)
    counts = np.bincount(
        np.asarray(ssc_target).reshape(-1).astype(np.int64), minlength=C
    )[:C]
    res = run_bass_kernel_spmd(nc, in_maps, core_ids=list(range(N_CORES)))
    outs = [np.asarray(r["out"], dtype=np.float32) for r in res.results]
    return _postprocess(outs, counts, f1_list).reshape(())


if __name__ == "__main__":
    rng = np.random.default_rng(0)
    pred = rng.standard_normal((2, C, 200, 200, 16), dtype=np.float32)
    tgt = rng.integers(0, C, size=(2, 200, 200, 16)).astype(np.int64)
    f1l = np.zeros((C,), np.float32)
    print(kernel(pred, tgt, f1l))
